# revision 45
# baseline (speedup 1.0000x reference)
"""Multi-head self-attention (B=4, T=2048, C=1024, H=16 heads, causal) on 8 TRN2
NeuronCores, head-tensor-parallel.

Per core c (owning heads 2c, 2c+1 = attn feature rows [c*128,(c+1)*128)):
  1. QKV per batch (bf16). q/k feature-major via weight-stationary groups.
     V is computed TOKEN-MAJOR directly (lhsT = x tile, rhs = w_v columns), so
     no PE transposes are needed; the column reversal the fp8 DoubleRow
     weight layout wants (hw row = 127 - col) happens in the DVE fp8 evac
     copies via a negative-stride input AP. Ones columns give the softmax
     denominator row for free.
  2. Causal attention: scoresT [kv, q] via row-tiled (K=64) bf16 matmul pairs
     (both heads in one 2-bank PSUM tile). exp(x*scale - 1) on ScalarE.
     Off-diagonal kv tiles: fp8 probs, PV via DoubleRow fp8 matmuls (two kv
     tiles per instruction). Diagonal kv tiles: bf16 probs, columns trimmed
     to the causal region, [128,128] triangle mask multiply, bf16 PV.
  3. Normalization: reciprocal_approx_fast of the denominator row,
     DMA-broadcast across partitions, DVE multiply straight out of PSUM.
  4. Interleaved emission: qkv(b) | attnE(b) | attnO(b-1) so ScalarE exp of
     one batch overlaps the next batch's QKV matmuls (TileScheduler reorders
     by readiness; per-batch q/k/v tiles keep the dependencies batch-local).
     Evens feed AllToAll 0; attnO(3) covers A2A0 + proj half 0; projection
     half 1 hides under the second AllToAll.
  5. Output projection (bf16) + bias for this core's 1024-token slice.
Host gathers the 8 [1024 feat, 1024 tok] slices, concatenates and transposes.
"""
import ml_dtypes
import numpy as np

import concourse.bass as bass
import concourse.tile as tile
from concourse import bacc, mybir
from concourse.bass_utils import run_bass_kernel_spmd

F32 = mybir.dt.float32
BF16 = mybir.dt.bfloat16
F8 = mybir.dt.float8e4

B, T, C = 4, 2048, 1024
N_HEADS, HEAD = 16, 64
N_CORES = 8
BT = B * T
TOK_PER_CORE = BT // N_CORES    # 1024
TB = 512                        # token block (matmul moving dim)
NKT = C // 128                  # 8 contraction tiles
SCALE = HEAD ** -0.5
EXP_BIAS = -1.0                 # exp(s*scale - 1); cancels in normalization


def build():
    nc = bacc.Bacc("TRN2", target_bir_lowering=False, debug=False, num_devices=N_CORES)

    xT = nc.dram_tensor("xT", [C, BT], BF16, kind="ExternalInput")
    wqkvT = nc.dram_tensor("wqkvT", [C, 384], BF16, kind="ExternalInput")
    wprojT = nc.dram_tensor("wprojT", [C, C], BF16, kind="ExternalInput")
    bmat = nc.dram_tensor("bmat", [128, 8], F32, kind="ExternalInput")
    tri_in = nc.dram_tensor("tri", [128, 128], BF16, kind="ExternalInput")

    outT = nc.dram_tensor("outT", [C, TOK_PER_CORE], F32, kind="ExternalOutput")

    # half X = even q-blocks (local token halves 0), half Y = odd q-blocks
    rnorm_d = nc.dram_tensor("rnorm_d", [16, 2 * TB], F32)
    a2i = [nc.dram_tensor(f"a2i{h}", [N_CORES, 128, TB], BF16) for h in range(2)]
    a2o = [nc.dram_tensor(f"a2o{h}", [N_CORES, 128, TB], BF16) for h in range(2)]

    xT_r = xT.ap().rearrange("(kt p) n -> p kt n", p=128)

    with tile.TileContext(nc) as tc:
        with (
            tc.tile_pool(name="consts", bufs=1) as consts,
            tc.tile_pool(name="qk", bufs=1) as qk_pool,
            tc.tile_pool(name="xt", bufs=3) as xt_pool,
            tc.tile_pool(name="exp", bufs=4) as exp_pool,
            tc.tile_pool(name="evac", bufs=3) as evac_pool,
            tc.tile_pool(name="sr", bufs=3) as sr_pool,
            tc.tile_pool(name="po", bufs=2) as po_pool,
            tc.tile_pool(name="ps_s", bufs=2, space="PSUM") as ps_s,   # scores
            tc.tile_pool(name="ps_pv", bufs=1, space="PSUM") as ps_pv,  # pv accum
            tc.tile_pool(name="ps_g", bufs=2, space="PSUM") as ps_g,   # qkv/v/proj
        ):
            wqkv_sb = consts.tile([128, NKT, 384], BF16)
            wqkvT_r = wqkvT.ap().rearrange("(kt p) m -> p kt m", p=128)
            for kt in range(NKT):
                nc.sync.dma_start(out=wqkv_sb[:, kt, :], in_=wqkvT_r[:, kt, :])
            tri_sb = consts.tile([128, 128], BF16)
            nc.sync.dma_start(out=tri_sb, in_=tri_in.ap())
            bmat_sb = consts.tile([128, 8], F32)
            nc.sync.dma_start(out=bmat_sb, in_=bmat.ap())
            ebias = consts.tile([128, 1], F32)
            nc.vector.memset(ebias[:], EXP_BIAS)
            wproj_sb = consts.tile([128, NKT, C], BF16)

            qTs, kTs, vns, v8s = [], [], [], []

            def rev_ap(ap_obj, length):
                # column-reversed view of a [P, length] AP (innermost stride -s)
                ap_list = [list(d) for d in ap_obj.ap]
                s = ap_list[-1][0]
                return bass.AP(
                    tensor=ap_obj.tensor,
                    offset=ap_obj.offset + s * (length - 1),
                    ap=ap_list[:-1] + [[-s, length]],
                )

            def qkv_batch(b):
                tok0 = b * T
                qT = qk_pool.tile([128, T], BF16, tag=f"q{b}", name=f"qT{b}")
                kT = qk_pool.tile([128, T], BF16, tag=f"k{b}", name=f"kT{b}")
                # v_nat: [onesA | A feats reversed | onesB | B feats reversed];
                # the diag PV lhsT walks it with stride -1 so out rows come out
                # natural, matching the fp8 DoubleRow row = 127-col mapping.
                v_nat = qk_pool.tile([128, 16, 130], BF16, tag=f"vn{b}", name=f"vn{b}")
                # interleaved dual-fp8 weight layout: [pair, col s, tile];
                # cols 64:128 hold host-reversed v features, col 63 ones
                vA8 = qk_pool.tile([128, 8, 128, 2], F8, tag=f"vA{b}", name=f"vA{b}")
                vB8 = qk_pool.tile([128, 8, 128, 2], F8, tag=f"vB{b}", name=f"vB{b}")
                nc.vector.memset(v_nat[:, :, 64], 1.0)
                nc.vector.memset(v_nat[:, :, 129], 1.0)
                for v8h in (vA8, vB8):
                    nc.vector.memset(v8h[:, :, 0:63, :], 0.0)
                    nc.vector.memset(v8h[:, :, 63, :], 1.0)
                qTs.append(qT); kTs.append(kT); vns.append(v_nat); v8s.append((vA8, vB8))

                for tb in range(T // TB):
                    col0 = tok0 + tb * TB
                    sl = slice(tb * TB, (tb + 1) * TB)
                    xt = xt_pool.tile([128, NKT, TB], BF16, tag="xt", name="xt")
                    for kt in range(NKT):
                        nc.sync.dma_start(out=xt[:, kt, :], in_=xT_r[:, kt, col0:col0 + TB])
                    for m in range(2):  # 0=q, 1=k (feature-major)
                        ps = ps_g.tile([128, TB], F32, tag="g", name="psg")
                        for kt in range(NKT):
                            nc.tensor.matmul(
                                ps[:],
                                lhsT=wqkv_sb[:, kt, m * 128:(m + 1) * 128],
                                rhs=xt[:, kt, :],
                                start=(kt == 0),
                                stop=(kt == NKT - 1),
                            )
                        if m == 0:
                            nc.vector.tensor_copy(qT[:, sl], ps[:])
                        else:
                            nc.vector.tensor_copy(kT[:, sl], ps[:])
                    # v token-major: lhsT = x tile (128 tokens), rhs = w_v cols
                    # (host-reversed per head, so fp8 tiles are plain copies)
                    for q4 in range(TB // 128):
                        jt = tb * 4 + q4
                        qsl = slice(q4 * 128, (q4 + 1) * 128)
                        psv = ps_g.tile([128, 128], F32, tag="g", name="psv")
                        for kt in range(NKT):
                            nc.tensor.matmul(
                                psv[:],
                                lhsT=xt[:, kt, qsl],
                                rhs=wqkv_sb[:, kt, 256:384],
                                start=(kt == 0),
                                stop=(kt == NKT - 1),
                            )
                        nc.vector.tensor_copy(v_nat[:, jt, 0:64], psv[:, 0:64])
                        nc.vector.tensor_copy(v_nat[:, jt, 65:129], psv[:, 64:128])
                        with nc.allow_low_precision(reason="off-diag PV in fp8"):
                            nc.vector.tensor_copy(
                                vA8[:, jt // 2, 64:128, jt % 2],
                                rev_ap(psv[:, 0:64], 64))
                            nc.vector.tensor_copy(
                                vB8[:, jt // 2, 64:128, jt % 2],
                                rev_ap(psv[:, 64:128], 64))

            # ---- causal attention, one q-block (TB tokens, both heads) ----
            def attn_block(b, ib, half):
                kT, v_nat = kTs[b], vns[b]
                vA8, vB8 = v8s[b]
                qt = qTs[b][:, ib * TB:(ib + 1) * TB]
                pv = ps_pv.tile([128, 2, TB], F32, tag="pv", name="pv")

                # off-diagonal kv tiles: fp8 probs, DoubleRow PV (2 kv tiles/mm)
                for pr in range(ib * 2):
                    jt0 = pr * 2
                    e2 = exp_pool.tile([128, 2, 2, TB], F8, tag="e2", name="e2")
                    for j in range(2):
                        jsl = slice((jt0 + j) * 128, (jt0 + j + 1) * 128)
                        s = ps_s.tile([128, 2, TB], F32, tag="s", name="s")
                        nc.tensor.matmul(
                            s[:, 0, :], lhsT=kT[0:64, jsl], rhs=qt[0:64, :],
                            start=True, stop=True, tile_position=(0, 0),
                        )
                        nc.tensor.matmul(
                            s[:, 1, :], lhsT=kT[64:128, jsl], rhs=qt[64:128, :],
                            start=True, stop=True, tile_position=(64, 0),
                        )
                        with nc.allow_low_precision(reason="off-diag probs in fp8"):
                            nc.scalar.activation(
                                e2[:, j, :, :], s[:],
                                mybir.ActivationFunctionType.Exp,
                                bias=ebias[:], scale=SCALE,
                            )
                    nc.tensor.matmul(
                        pv[0:128, 0, :], lhsT=vA8[:, pr, :, :],
                        rhs=e2[:, :, 0, :],
                        start=(pr == 0), stop=False,
                        perf_mode=mybir.MatmulPerfMode.DoubleRowSwInterleave,
                        skip_group_check=True,
                    )
                    nc.tensor.matmul(
                        pv[0:128, 1, :], lhsT=vB8[:, pr, :, :],
                        rhs=e2[:, :, 1, :],
                        start=(pr == 0), stop=False,
                        perf_mode=mybir.MatmulPerfMode.DoubleRowSwInterleave,
                        skip_group_check=True,
                    )

                # diagonal kv tiles: bf16 probs, causal column trim + triangle mask
                for jl in range(4):
                    jt = ib * 4 + jl
                    c0 = jl * 128
                    jsl = slice(jt * 128, (jt + 1) * 128)
                    s = ps_s.tile([128, 2, TB], F32, tag="s", name="sd")
                    nc.tensor.matmul(
                        s[:, 0, c0:TB], lhsT=kT[0:64, jsl], rhs=qt[0:64, c0:TB],
                        start=True, stop=True, tile_position=(0, 0),
                    )
                    nc.tensor.matmul(
                        s[:, 1, c0:TB], lhsT=kT[64:128, jsl], rhs=qt[64:128, c0:TB],
                        start=True, stop=True, tile_position=(64, 0),
                    )
                    e = exp_pool.tile([128, 2, TB], BF16, tag="e", name="e")
                    nc.scalar.activation(
                        e[:, :, c0:TB], s[:, :, c0:TB],
                        mybir.ActivationFunctionType.Exp,
                        bias=ebias[:], scale=SCALE,
                    )
                    with nc.allow_low_precision(reason="exact 0/1 mask on bf16 probs"):
                        nc.vector.tensor_mul(
                            e[:, 0, c0:c0 + 128], e[:, 0, c0:c0 + 128], tri_sb[:])
                        nc.vector.tensor_mul(
                            e[:, 1, c0:c0 + 128], e[:, 1, c0:c0 + 128], tri_sb[:])
                    first = (ib == 0 and jl == 0)
                    nc.tensor.matmul(
                        pv[0:65, 0, c0:TB], lhsT=v_nat[:, jt, 0:65],
                        rhs=e[:, 0, c0:TB],
                        start=first, stop=(jl == 3), skip_group_check=True,
                    )
                    nc.tensor.matmul(
                        pv[0:65, 1, c0:TB], lhsT=v_nat[:, jt, 65:130],
                        rhs=e[:, 1, c0:TB],
                        start=first, stop=(jl == 3), skip_group_check=True,
                    )

                # normalize both heads and ship to the AG input for this half
                blk = half * 8 + b * 2 + ib // 2
                srow = sr_pool.tile([1, 2, TB], F32, tag="sr", name="srow")
                nc.vector.tensor_copy(srow[:], pv[64:65, :, :])
                r32 = sr_pool.tile([1, 2, TB], F32, tag="sr", name="r32")
                nc.vector.reciprocal_approx_fast(out=r32[:], in_=srow[:])
                nc.sync.dma_start(out=rnorm_d.ap()[blk, :], in_=r32[:])
                rb = evac_pool.tile([64, 2, TB], F32, tag="rb", name="rb")
                base = rnorm_d.ap()[blk, :]
                rb_src = bass.AP(
                    tensor=base.tensor,
                    offset=base.offset,
                    ap=[[0, 64], [TB, 2], [1, TB]],
                )
                nc.sync.dma_start(out=rb[:], in_=rb_src)
                outn = evac_pool.tile([64, 2, TB], BF16, tag="on", name="outn")
                with nc.allow_low_precision(reason="normalized attn out as bf16"):
                    nc.vector.tensor_mul(outn[:], pv[0:64, :, :], rb[:])
                chunk = b * 2 + ib // 2
                nc.gpsimd.dma_start(out=a2i[half].ap()[chunk, 0:64, :], in_=outn[:, 0, :])
                nc.gpsimd.dma_start(out=a2i[half].ap()[chunk, 64:128, :], in_=outn[:, 1, :])

            # ---- interleaved emission: qkv(b) | attnE(b) | attnO(b-1) ----
            for b in range(B):
                qkv_batch(b)
                if b == 0:
                    nc.sync.dma_start(
                        out=wproj_sb,
                        in_=wprojT.ap().rearrange("(kt p) m -> p kt m", p=128),
                    )
                for ib in (0, 2):
                    attn_block(b, ib, 0)
                if b >= 1:
                    for ib in (1, 3):
                        attn_block(b - 1, ib, 1)
            nc.gpsimd.collective_compute(
                "AllToAll", mybir.AluOpType.bypass,
                ins=[a2i[0].ap()], outs=[a2o[0].ap()],
                replica_groups=[list(range(N_CORES))],
            )
            for ib in (3, 1):
                attn_block(B - 1, ib, 1)

            # ---- output projection for my 1024-token slice ----
            for half in range(2):
                if half == 1:
                    nc.gpsimd.collective_compute(
                        "AllToAll", mybir.AluOpType.bypass,
                        ins=[a2i[1].ap()], outs=[a2o[1].ap()],
                        replica_groups=[list(range(N_CORES))],
                    )
                at = xt_pool.tile([128, NKT, TB], BF16, tag="at", name="at")
                for kt in range(NKT):
                    nc.sync.dma_start(
                        out=at[:, kt, :],
                        in_=a2o[half].ap()[kt, :, :],
                    )
                for dt in range(8):
                    ps = ps_g.tile([128, TB], F32, tag="g", name="psp")
                    for kt in range(NKT):
                        nc.tensor.matmul(
                            ps[:],
                            lhsT=wproj_sb[:, kt, dt * 128:(dt + 1) * 128],
                            rhs=at[:, kt, :],
                            start=(kt == 0),
                            stop=(kt == NKT - 1),
                        )
                    ot = po_pool.tile([128, TB], F32, tag="po", name="ot")
                    nc.scalar.activation(
                        ot[:], ps[:], mybir.ActivationFunctionType.Identity,
                        bias=bmat_sb[:, dt:dt + 1], scale=1.0,
                    )
                    nc.sync.dma_start(
                        out=outT.ap()[dt * 128:(dt + 1) * 128, half * TB:(half + 1) * TB],
                        in_=ot[:],
                    )

    nc.compile()
    return nc


_NC = None
_last_in_maps = None


def _get_nc():
    global _NC
    if _NC is None:
        _NC = build()
    return _NC


def kernel(x, w_qkv, w_proj, b_proj):
    nc = _get_nc()

    x = np.asarray(x, dtype=np.float32)
    w_qkv = np.asarray(w_qkv, dtype=np.float32)
    w_proj = np.asarray(w_proj, dtype=np.float32)
    b_proj = np.asarray(b_proj, dtype=np.float32)

    xT = np.ascontiguousarray(x.reshape(BT, C).T).astype(ml_dtypes.bfloat16)
    bmat = np.ascontiguousarray(b_proj.reshape(8, 128).T)
    p = np.arange(128)[:, None]
    f = np.arange(128)[None, :]
    tri = (p <= f).astype(ml_dtypes.bfloat16)

    # v features are computed reversed within each head so the fp8 DoubleRow
    # weight tiles (hw row = 127 - col) come out natural; the diag PV walks
    # v_nat with stride -1 to match. w_proj needs no permutation.
    wprojT = np.ascontiguousarray(w_proj.T).astype(ml_dtypes.bfloat16)

    in_maps = []
    for c in range(N_CORES):
        rows = slice(c * 128, (c + 1) * 128)
        w_v = w_qkv[2 * C:3 * C][rows]
        w_local = np.concatenate(
            [w_qkv[0:C][rows], w_qkv[C:2 * C][rows], w_v], axis=0
        )  # [384, C]
        in_maps.append({
            "xT": xT,
            "wqkvT": np.ascontiguousarray(w_local.T).astype(ml_dtypes.bfloat16),
            "wprojT": wprojT,
            "bmat": bmat,
            "tri": tri,
        })

    global _last_in_maps
    _last_in_maps = in_maps
    res = run_bass_kernel_spmd(nc, in_maps, core_ids=list(range(N_CORES)))
    outT_full = np.concatenate([res.results[c]["outT"] for c in range(N_CORES)], axis=1)
    return np.ascontiguousarray(outT_full.T).reshape(B, T, C)


# revision 46
# speedup vs baseline: 1.0110x; 1.0110x over previous
"""Multi-head self-attention (B=4, T=2048, C=1024, H=16 heads, causal) on 8 TRN2
NeuronCores, head-tensor-parallel.

Per core c (owning heads 2c, 2c+1 = attn feature rows [c*128,(c+1)*128)):
  1. QKV per batch (bf16). q/k feature-major via weight-stationary groups.
     V is computed TOKEN-MAJOR directly (lhsT = x tile, rhs = w_v columns), so
     no PE transposes are needed; the column reversal the fp8 DoubleRow
     weight layout wants (hw row = 127 - col) happens in the DVE fp8 evac
     copies via a negative-stride input AP. Ones columns give the softmax
     denominator row for free.
  2. Causal attention: scoresT [kv, q] via row-tiled (K=64) bf16 matmul pairs
     (both heads in one 2-bank PSUM tile). exp(x*scale - 1) on ScalarE.
     Off-diagonal kv tiles: fp8 probs, PV via DoubleRow fp8 matmuls (two kv
     tiles per instruction). Diagonal kv tiles: bf16 probs, columns trimmed
     to the causal region, [128,128] triangle mask multiply, bf16 PV.
  3. Normalization: reciprocal_approx_fast of the denominator row,
     DMA-broadcast across partitions, DVE multiply straight out of PSUM.
  4. Interleaved emission: qkv(b) | attnE(b) | attnO(b-1) so ScalarE exp of
     one batch overlaps the next batch's QKV matmuls (TileScheduler reorders
     by readiness; per-batch q/k/v tiles keep the dependencies batch-local).
     Evens feed AllToAll 0; attnO(3) covers A2A0 + proj half 0; projection
     half 1 hides under the second AllToAll.
  5. Output projection (bf16) + bias for this core's 1024-token slice.
Host gathers the 8 [1024 feat, 1024 tok] slices, concatenates and transposes.
"""
import ml_dtypes
import numpy as np

import concourse.bass as bass
import concourse.tile as tile
from concourse import bacc, mybir
from concourse.bass_utils import run_bass_kernel_spmd

F32 = mybir.dt.float32
BF16 = mybir.dt.bfloat16
F8 = mybir.dt.float8e4

B, T, C = 4, 2048, 1024
N_HEADS, HEAD = 16, 64
N_CORES = 8
BT = B * T
TOK_PER_CORE = BT // N_CORES    # 1024
TB = 512                        # token block (matmul moving dim)
NKT = C // 128                  # 8 contraction tiles
SCALE = HEAD ** -0.5
EXP_BIAS = -1.0                 # exp(s*scale - 1); cancels in normalization


def build():
    nc = bacc.Bacc("TRN2", target_bir_lowering=False, debug=False, num_devices=N_CORES)

    xT = nc.dram_tensor("xT", [C, BT], BF16, kind="ExternalInput")
    wqkvT = nc.dram_tensor("wqkvT", [C, 384], BF16, kind="ExternalInput")
    wprojT = nc.dram_tensor("wprojT", [C, C], BF16, kind="ExternalInput")
    bmat = nc.dram_tensor("bmat", [128, 8], F32, kind="ExternalInput")
    tri_in = nc.dram_tensor("tri", [128, 128], BF16, kind="ExternalInput")

    outT = nc.dram_tensor("outT", [C, TOK_PER_CORE], F32, kind="ExternalOutput")

    # half X = even q-blocks (local token halves 0), half Y = odd q-blocks
    rnorm_d = nc.dram_tensor("rnorm_d", [16, 2 * TB], F32)
    a2i = [nc.dram_tensor(f"a2i{h}", [N_CORES, 128, TB], BF16) for h in range(2)]
    a2o = [nc.dram_tensor(f"a2o{h}", [N_CORES, 128, TB], BF16) for h in range(2)]

    xT_r = xT.ap().rearrange("(kt p) n -> p kt n", p=128)

    with tile.TileContext(nc) as tc:
        with (
            tc.tile_pool(name="consts", bufs=1) as consts,
            tc.tile_pool(name="qk", bufs=1) as qk_pool,
            tc.tile_pool(name="xt", bufs=3) as xt_pool,
            tc.tile_pool(name="exp", bufs=4) as exp_pool,
            tc.tile_pool(name="evac", bufs=3) as evac_pool,
            tc.tile_pool(name="sr", bufs=3) as sr_pool,
            tc.tile_pool(name="po", bufs=2) as po_pool,
            tc.tile_pool(name="ps_s", bufs=2, space="PSUM") as ps_s,   # scores
            tc.tile_pool(name="ps_pv", bufs=1, space="PSUM") as ps_pv,  # pv accum
            tc.tile_pool(name="ps_g", bufs=2, space="PSUM") as ps_g,   # qkv/v/proj
        ):
            wqkv_sb = consts.tile([128, NKT, 384], BF16)
            wqkvT_r = wqkvT.ap().rearrange("(kt p) m -> p kt m", p=128)
            for kt in range(NKT):
                nc.sync.dma_start(out=wqkv_sb[:, kt, :], in_=wqkvT_r[:, kt, :])
            tri_sb = consts.tile([128, 128], BF16)
            nc.sync.dma_start(out=tri_sb, in_=tri_in.ap())
            bmat_sb = consts.tile([128, 8], F32)
            nc.sync.dma_start(out=bmat_sb, in_=bmat.ap())
            ebias = consts.tile([128, 1], F32)
            nc.vector.memset(ebias[:], EXP_BIAS)
            wproj_sb = consts.tile([128, NKT, C], BF16)

            qTs, kTs, vns, v8s = [], [], [], []

            def rev_ap(ap_obj, length):
                # column-reversed view of a [P, length] AP (innermost stride -s)
                ap_list = [list(d) for d in ap_obj.ap]
                s = ap_list[-1][0]
                return bass.AP(
                    tensor=ap_obj.tensor,
                    offset=ap_obj.offset + s * (length - 1),
                    ap=ap_list[:-1] + [[-s, length]],
                )

            def qkv_batch(b):
                tok0 = b * T
                qT = qk_pool.tile([128, T], BF16, tag=f"q{b}", name=f"qT{b}")
                kT = qk_pool.tile([128, T], BF16, tag=f"k{b}", name=f"kT{b}")
                # v_nat: [onesA | A feats reversed | onesB | B feats reversed];
                # the diag PV lhsT walks it with stride -1 so out rows come out
                # natural, matching the fp8 DoubleRow row = 127-col mapping.
                v_nat = qk_pool.tile([128, 16, 130], BF16, tag=f"vn{b}", name=f"vn{b}")
                # interleaved dual-fp8 weight layout: [pair, col s, tile];
                # cols 64:128 hold host-reversed v features, col 63 ones
                vA8 = qk_pool.tile([128, 8, 128, 2], F8, tag=f"vA{b}", name=f"vA{b}")
                vB8 = qk_pool.tile([128, 8, 128, 2], F8, tag=f"vB{b}", name=f"vB{b}")
                nc.vector.memset(v_nat[:, :, 64], 1.0)
                nc.vector.memset(v_nat[:, :, 129], 1.0)
                for v8h in (vA8, vB8):
                    nc.vector.memset(v8h[:, :, 0:63, :], 0.0)
                    nc.vector.memset(v8h[:, :, 63, :], 1.0)
                qTs.append(qT); kTs.append(kT); vns.append(v_nat); v8s.append((vA8, vB8))

                for tb in range(T // TB):
                    col0 = tok0 + tb * TB
                    sl = slice(tb * TB, (tb + 1) * TB)
                    xt = xt_pool.tile([128, NKT, TB], BF16, tag="xt", name="xt")
                    for kt in range(NKT):
                        nc.sync.dma_start(out=xt[:, kt, :], in_=xT_r[:, kt, col0:col0 + TB])
                    for m in range(2):  # 0=q, 1=k (feature-major)
                        ps = ps_g.tile([128, TB], F32, tag="g", name="psg")
                        for kt in range(NKT):
                            nc.tensor.matmul(
                                ps[:],
                                lhsT=wqkv_sb[:, kt, m * 128:(m + 1) * 128],
                                rhs=xt[:, kt, :],
                                start=(kt == 0),
                                stop=(kt == NKT - 1),
                            )
                        if m == 0:
                            nc.vector.tensor_copy(qT[:, sl], ps[:])
                        else:
                            nc.vector.tensor_copy(kT[:, sl], ps[:])
                    # v token-major: lhsT = x tile (128 tokens), rhs = w_v cols
                    # (host-reversed per head, so fp8 tiles are plain copies)
                    for q4 in range(TB // 128):
                        jt = tb * 4 + q4
                        qsl = slice(q4 * 128, (q4 + 1) * 128)
                        psv = ps_g.tile([128, 128], F32, tag="g", name="psv")
                        for kt in range(NKT):
                            nc.tensor.matmul(
                                psv[:],
                                lhsT=xt[:, kt, qsl],
                                rhs=wqkv_sb[:, kt, 256:384],
                                start=(kt == 0),
                                stop=(kt == NKT - 1),
                            )
                        nc.vector.tensor_copy(v_nat[:, jt, 0:64], psv[:, 0:64])
                        nc.vector.tensor_copy(v_nat[:, jt, 65:129], psv[:, 64:128])
                        with nc.allow_low_precision(reason="off-diag PV in fp8"):
                            nc.vector.tensor_copy(
                                vA8[:, jt // 2, 64:128, jt % 2],
                                rev_ap(psv[:, 0:64], 64))
                            nc.vector.tensor_copy(
                                vB8[:, jt // 2, 64:128, jt % 2],
                                rev_ap(psv[:, 64:128], 64))

            # ---- causal attention, one q-block (TB tokens, both heads) ----
            def attn_block(b, ib, half):
                kT, v_nat = kTs[b], vns[b]
                vA8, vB8 = v8s[b]
                qt = qTs[b][:, ib * TB:(ib + 1) * TB]
                pv = ps_pv.tile([128, 2, TB], F32, tag="pv", name="pv")

                # off-diagonal kv tiles: fp8 probs, DoubleRow PV (2 kv tiles/mm)
                for pr in range(ib * 2):
                    jt0 = pr * 2
                    e2 = exp_pool.tile([128, 2, 2, TB], F8, tag="e2", name="e2")
                    for j in range(2):
                        jsl = slice((jt0 + j) * 128, (jt0 + j + 1) * 128)
                        s = ps_s.tile([128, 2, TB], F32, tag="s", name="s")
                        nc.tensor.matmul(
                            s[:, 0, :], lhsT=kT[0:64, jsl], rhs=qt[0:64, :],
                            start=True, stop=True, tile_position=(0, 0),
                        )
                        nc.tensor.matmul(
                            s[:, 1, :], lhsT=kT[64:128, jsl], rhs=qt[64:128, :],
                            start=True, stop=True, tile_position=(64, 0),
                        )
                        with nc.allow_low_precision(reason="off-diag probs in fp8"):
                            nc.scalar.activation(
                                e2[:, j, :, :], s[:],
                                mybir.ActivationFunctionType.Exp,
                                bias=ebias[:], scale=SCALE,
                            )
                    nc.tensor.matmul(
                        pv[0:128, 0, :], lhsT=vA8[:, pr, :, :],
                        rhs=e2[:, :, 0, :],
                        start=(pr == 0), stop=False,
                        perf_mode=mybir.MatmulPerfMode.DoubleRowSwInterleave,
                        skip_group_check=True,
                    )
                    nc.tensor.matmul(
                        pv[0:128, 1, :], lhsT=vB8[:, pr, :, :],
                        rhs=e2[:, :, 1, :],
                        start=(pr == 0), stop=False,
                        perf_mode=mybir.MatmulPerfMode.DoubleRowSwInterleave,
                        skip_group_check=True,
                    )

                # diagonal kv tiles: bf16 probs, causal column trim + triangle mask
                for jl in range(4):
                    jt = ib * 4 + jl
                    c0 = jl * 128
                    jsl = slice(jt * 128, (jt + 1) * 128)
                    s = ps_s.tile([128, 2, TB], F32, tag="s", name="sd")
                    nc.tensor.matmul(
                        s[:, 0, c0:TB], lhsT=kT[0:64, jsl], rhs=qt[0:64, c0:TB],
                        start=True, stop=True, tile_position=(0, 0),
                    )
                    nc.tensor.matmul(
                        s[:, 1, c0:TB], lhsT=kT[64:128, jsl], rhs=qt[64:128, c0:TB],
                        start=True, stop=True, tile_position=(64, 0),
                    )
                    e = exp_pool.tile([128, 2, TB], BF16, tag="e", name="e")
                    nc.scalar.activation(
                        e[:, :, c0:TB], s[:, :, c0:TB],
                        mybir.ActivationFunctionType.Exp,
                        bias=ebias[:], scale=SCALE,
                    )
                    with nc.allow_low_precision(reason="exact 0/1 mask on bf16 probs"):
                        nc.vector.tensor_mul(
                            e[:, 0, c0:c0 + 128], e[:, 0, c0:c0 + 128], tri_sb[:])
                        nc.vector.tensor_mul(
                            e[:, 1, c0:c0 + 128], e[:, 1, c0:c0 + 128], tri_sb[:])
                    first = (ib == 0 and jl == 0)
                    nc.tensor.matmul(
                        pv[0:65, 0, c0:TB], lhsT=v_nat[:, jt, 0:65],
                        rhs=e[:, 0, c0:TB],
                        start=first, stop=(jl == 3), skip_group_check=True,
                    )
                    nc.tensor.matmul(
                        pv[0:65, 1, c0:TB], lhsT=v_nat[:, jt, 65:130],
                        rhs=e[:, 1, c0:TB],
                        start=first, stop=(jl == 3), skip_group_check=True,
                    )

                # normalize both heads and ship to the AG input for this half
                blk = half * 8 + b * 2 + ib // 2
                srow = sr_pool.tile([1, 2, TB], F32, tag="sr", name="srow")
                nc.vector.tensor_copy(srow[:], pv[64:65, :, :])
                r32 = sr_pool.tile([1, 2, TB], F32, tag="sr", name="r32")
                nc.vector.reciprocal_approx_fast(out=r32[:], in_=srow[:])
                rb = evac_pool.tile([64, 2, TB], F32, tag="rb", name="rb")
                nc.gpsimd.partition_broadcast(rb[:], r32[:])
                outn = evac_pool.tile([64, 2, TB], BF16, tag="on", name="outn")
                with nc.allow_low_precision(reason="normalized attn out as bf16"):
                    nc.vector.tensor_mul(outn[:], pv[0:64, :, :], rb[:])
                chunk = b * 2 + ib // 2
                nc.gpsimd.dma_start(out=a2i[half].ap()[chunk, 0:64, :], in_=outn[:, 0, :])
                nc.gpsimd.dma_start(out=a2i[half].ap()[chunk, 64:128, :], in_=outn[:, 1, :])

            # ---- interleaved emission: qkv(b) | attnE(b) | attnO(b-1) ----
            for b in range(B):
                qkv_batch(b)
                if b == 0:
                    nc.sync.dma_start(
                        out=wproj_sb,
                        in_=wprojT.ap().rearrange("(kt p) m -> p kt m", p=128),
                    )
                for ib in (0, 2):
                    attn_block(b, ib, 0)
                if b >= 1:
                    for ib in (1, 3):
                        attn_block(b - 1, ib, 1)
            nc.gpsimd.collective_compute(
                "AllToAll", mybir.AluOpType.bypass,
                ins=[a2i[0].ap()], outs=[a2o[0].ap()],
                replica_groups=[list(range(N_CORES))],
            )
            for ib in (3, 1):
                attn_block(B - 1, ib, 1)

            # ---- output projection for my 1024-token slice ----
            for half in range(2):
                if half == 1:
                    nc.gpsimd.collective_compute(
                        "AllToAll", mybir.AluOpType.bypass,
                        ins=[a2i[1].ap()], outs=[a2o[1].ap()],
                        replica_groups=[list(range(N_CORES))],
                    )
                at = xt_pool.tile([128, NKT, TB], BF16, tag="at", name="at")
                for kt in range(NKT):
                    nc.sync.dma_start(
                        out=at[:, kt, :],
                        in_=a2o[half].ap()[kt, :, :],
                    )
                for dt in range(8):
                    ps = ps_g.tile([128, TB], F32, tag="g", name="psp")
                    for kt in range(NKT):
                        nc.tensor.matmul(
                            ps[:],
                            lhsT=wproj_sb[:, kt, dt * 128:(dt + 1) * 128],
                            rhs=at[:, kt, :],
                            start=(kt == 0),
                            stop=(kt == NKT - 1),
                        )
                    ot = po_pool.tile([128, TB], F32, tag="po", name="ot")
                    nc.scalar.activation(
                        ot[:], ps[:], mybir.ActivationFunctionType.Identity,
                        bias=bmat_sb[:, dt:dt + 1], scale=1.0,
                    )
                    nc.sync.dma_start(
                        out=outT.ap()[dt * 128:(dt + 1) * 128, half * TB:(half + 1) * TB],
                        in_=ot[:],
                    )

    nc.compile()
    return nc


_NC = None
_last_in_maps = None


def _get_nc():
    global _NC
    if _NC is None:
        _NC = build()
    return _NC


def kernel(x, w_qkv, w_proj, b_proj):
    nc = _get_nc()

    x = np.asarray(x, dtype=np.float32)
    w_qkv = np.asarray(w_qkv, dtype=np.float32)
    w_proj = np.asarray(w_proj, dtype=np.float32)
    b_proj = np.asarray(b_proj, dtype=np.float32)

    xT = np.ascontiguousarray(x.reshape(BT, C).T).astype(ml_dtypes.bfloat16)
    bmat = np.ascontiguousarray(b_proj.reshape(8, 128).T)
    p = np.arange(128)[:, None]
    f = np.arange(128)[None, :]
    tri = (p <= f).astype(ml_dtypes.bfloat16)

    # v features are computed reversed within each head so the fp8 DoubleRow
    # weight tiles (hw row = 127 - col) come out natural; the diag PV walks
    # v_nat with stride -1 to match. w_proj needs no permutation.
    wprojT = np.ascontiguousarray(w_proj.T).astype(ml_dtypes.bfloat16)

    in_maps = []
    for c in range(N_CORES):
        rows = slice(c * 128, (c + 1) * 128)
        w_v = w_qkv[2 * C:3 * C][rows]
        w_local = np.concatenate(
            [w_qkv[0:C][rows], w_qkv[C:2 * C][rows], w_v], axis=0
        )  # [384, C]
        in_maps.append({
            "xT": xT,
            "wqkvT": np.ascontiguousarray(w_local.T).astype(ml_dtypes.bfloat16),
            "wprojT": wprojT,
            "bmat": bmat,
            "tri": tri,
        })

    global _last_in_maps
    _last_in_maps = in_maps
    res = run_bass_kernel_spmd(nc, in_maps, core_ids=list(range(N_CORES)))
    outT_full = np.concatenate([res.results[c]["outT"] for c in range(N_CORES)], axis=1)
    return np.ascontiguousarray(outT_full.T).reshape(B, T, C)


# revision 47
# speedup vs baseline: 1.0455x; 1.0341x over previous
"""Multi-head self-attention (B=4, T=2048, C=1024, H=16 heads, causal) on 8 TRN2
NeuronCores, head-tensor-parallel.

Per core c (owning heads 2c, 2c+1 = attn feature rows [c*128,(c+1)*128)):
  1. QKV per batch (bf16). q/k feature-major via weight-stationary groups.
     V is computed TOKEN-MAJOR directly (lhsT = x tile, rhs = w_v columns), so
     no PE transposes are needed; the column reversal the fp8 DoubleRow
     weight layout wants (hw row = 127 - col) happens in the DVE fp8 evac
     copies via a negative-stride input AP. Ones columns give the softmax
     denominator row for free.
  2. Causal attention: scoresT [kv, q] via row-tiled (K=64) bf16 matmul pairs
     (both heads in one 2-bank PSUM tile). exp(x*scale - 1) on ScalarE.
     Off-diagonal kv tiles: fp8 probs, PV via DoubleRow fp8 matmuls (two kv
     tiles per instruction). Diagonal kv tiles: bf16 probs, columns trimmed
     to the causal region, [128,128] triangle mask multiply, bf16 PV.
  3. Normalization: reciprocal_approx_fast of the denominator row,
     DMA-broadcast across partitions, DVE multiply straight out of PSUM.
  4. Interleaved emission: qkv(b) | attnE(b) | attnO(b-1) so ScalarE exp of
     one batch overlaps the next batch's QKV matmuls (TileScheduler reorders
     by readiness; per-batch q/k/v tiles keep the dependencies batch-local).
     Evens feed AllToAll 0; attnO(3) covers A2A0 + proj half 0; projection
     half 1 hides under the second AllToAll.
  5. Output projection (bf16) + bias for this core's 1024-token slice.
Host gathers the 8 [1024 feat, 1024 tok] slices, concatenates and transposes.
"""
import ml_dtypes
import numpy as np

import concourse.bass as bass
import concourse.tile as tile
from concourse import bacc, mybir
from concourse.bass_utils import run_bass_kernel_spmd

F32 = mybir.dt.float32
BF16 = mybir.dt.bfloat16
F8 = mybir.dt.float8e4

B, T, C = 4, 2048, 1024
N_HEADS, HEAD = 16, 64
N_CORES = 8
BT = B * T
TOK_PER_CORE = BT // N_CORES    # 1024
TB = 512                        # token block (matmul moving dim)
NKT = C // 128                  # 8 contraction tiles
SCALE = HEAD ** -0.5
EXP_BIAS = -1.0                 # exp(s*scale - 1); cancels in normalization


def build():
    nc = bacc.Bacc("TRN2", target_bir_lowering=False, debug=False, num_devices=N_CORES)

    xT = nc.dram_tensor("xT", [C, BT], BF16, kind="ExternalInput")
    wqkvT = nc.dram_tensor("wqkvT", [C, 384], BF16, kind="ExternalInput")
    wprojT = nc.dram_tensor("wprojT", [C, C], BF16, kind="ExternalInput")
    bmat = nc.dram_tensor("bmat", [128, 8], F32, kind="ExternalInput")
    tri_in = nc.dram_tensor("tri", [128, 128], BF16, kind="ExternalInput")

    outT = nc.dram_tensor("outT", [C, TOK_PER_CORE], F32, kind="ExternalOutput")

    # half X = even q-blocks (local token halves 0), half Y = odd q-blocks
    rnorm_d = nc.dram_tensor("rnorm_d", [16, 2 * TB], F32)
    a2i = [nc.dram_tensor(f"a2i{h}", [N_CORES, 128, TB], BF16) for h in range(2)]
    a2o = [nc.dram_tensor(f"a2o{h}", [N_CORES, 128, TB], BF16) for h in range(2)]

    xT_r = xT.ap().rearrange("(kt p) n -> p kt n", p=128)

    with tile.TileContext(nc) as tc:
        with (
            tc.tile_pool(name="consts", bufs=1) as consts,
            tc.tile_pool(name="qk", bufs=1) as qk_pool,
            tc.tile_pool(name="xt", bufs=3) as xt_pool,
            tc.tile_pool(name="exp", bufs=6) as exp_pool,
            tc.tile_pool(name="evac", bufs=3) as evac_pool,
            tc.tile_pool(name="sr", bufs=3) as sr_pool,
            tc.tile_pool(name="po", bufs=2) as po_pool,
            tc.tile_pool(name="ps_s", bufs=2, space="PSUM") as ps_s,   # scores
            tc.tile_pool(name="ps_pv", bufs=1, space="PSUM") as ps_pv,  # pv accum
            tc.tile_pool(name="ps_g", bufs=2, space="PSUM") as ps_g,   # qkv/v/proj
        ):
            wqkv_sb = consts.tile([128, NKT, 384], BF16)
            wqkvT_r = wqkvT.ap().rearrange("(kt p) m -> p kt m", p=128)
            for kt in range(NKT):
                nc.sync.dma_start(out=wqkv_sb[:, kt, :], in_=wqkvT_r[:, kt, :])
            tri_sb = consts.tile([128, 128], BF16)
            nc.sync.dma_start(out=tri_sb, in_=tri_in.ap())
            bmat_sb = consts.tile([128, 8], F32)
            nc.sync.dma_start(out=bmat_sb, in_=bmat.ap())
            ebias = consts.tile([128, 1], F32)
            nc.vector.memset(ebias[:], EXP_BIAS)
            wproj_sb = consts.tile([128, NKT, C], BF16)

            qTs, kTs, vns, v8s = [], [], [], []

            def rev_ap(ap_obj, length):
                # column-reversed view of a [P, length] AP (innermost stride -s)
                ap_list = [list(d) for d in ap_obj.ap]
                s = ap_list[-1][0]
                return bass.AP(
                    tensor=ap_obj.tensor,
                    offset=ap_obj.offset + s * (length - 1),
                    ap=ap_list[:-1] + [[-s, length]],
                )

            def qkv_batch(b):
                tok0 = b * T
                qT = qk_pool.tile([128, T], BF16, tag=f"q{b}", name=f"qT{b}")
                kT = qk_pool.tile([128, T], BF16, tag=f"k{b}", name=f"kT{b}")
                # v_nat: [onesA | A feats reversed | onesB | B feats reversed];
                # the diag PV lhsT walks it with stride -1 so out rows come out
                # natural, matching the fp8 DoubleRow row = 127-col mapping.
                v_nat = qk_pool.tile([128, 16, 130], BF16, tag=f"vn{b}", name=f"vn{b}")
                # interleaved dual-fp8 weight layout: [pair, col s, tile];
                # cols 64:128 hold host-reversed v features, col 63 ones
                vA8 = qk_pool.tile([128, 8, 128, 2], F8, tag=f"vA{b}", name=f"vA{b}")
                vB8 = qk_pool.tile([128, 8, 128, 2], F8, tag=f"vB{b}", name=f"vB{b}")
                nc.vector.memset(v_nat[:, :, 64], 1.0)
                nc.vector.memset(v_nat[:, :, 129], 1.0)
                for v8h in (vA8, vB8):
                    nc.vector.memset(v8h[:, :, 0:63, :], 0.0)
                    nc.vector.memset(v8h[:, :, 63, :], 1.0)
                qTs.append(qT); kTs.append(kT); vns.append(v_nat); v8s.append((vA8, vB8))

                for tb in range(T // TB):
                    col0 = tok0 + tb * TB
                    sl = slice(tb * TB, (tb + 1) * TB)
                    xt = xt_pool.tile([128, NKT, TB], BF16, tag="xt", name="xt")
                    for kt in range(NKT):
                        nc.sync.dma_start(out=xt[:, kt, :], in_=xT_r[:, kt, col0:col0 + TB])
                    for m in range(2):  # 0=q, 1=k (feature-major)
                        ps = ps_g.tile([128, TB], F32, tag="g", name="psg")
                        for kt in range(NKT):
                            nc.tensor.matmul(
                                ps[:],
                                lhsT=wqkv_sb[:, kt, m * 128:(m + 1) * 128],
                                rhs=xt[:, kt, :],
                                start=(kt == 0),
                                stop=(kt == NKT - 1),
                            )
                        if m == 0:
                            nc.vector.tensor_copy(qT[:, sl], ps[:])
                        else:
                            nc.vector.tensor_copy(kT[:, sl], ps[:])
                    # v token-major: lhsT = x tile (128 tokens), rhs = w_v cols
                    # (host-reversed per head, so fp8 tiles are plain copies)
                    for q4 in range(TB // 128):
                        jt = tb * 4 + q4
                        qsl = slice(q4 * 128, (q4 + 1) * 128)
                        psv = ps_g.tile([128, 128], F32, tag="g", name="psv")
                        for kt in range(NKT):
                            nc.tensor.matmul(
                                psv[:],
                                lhsT=xt[:, kt, qsl],
                                rhs=wqkv_sb[:, kt, 256:384],
                                start=(kt == 0),
                                stop=(kt == NKT - 1),
                            )
                        nc.vector.tensor_copy(v_nat[:, jt, 0:64], psv[:, 0:64])
                        nc.vector.tensor_copy(v_nat[:, jt, 65:129], psv[:, 64:128])
                        with nc.allow_low_precision(reason="off-diag PV in fp8"):
                            nc.vector.tensor_copy(
                                vA8[:, jt // 2, 64:128, jt % 2],
                                rev_ap(psv[:, 0:64], 64))
                            nc.vector.tensor_copy(
                                vB8[:, jt // 2, 64:128, jt % 2],
                                rev_ap(psv[:, 64:128], 64))

            # ---- causal attention, one q-block (TB tokens, both heads) ----
            def attn_block(b, ib, half):
                kT, v_nat = kTs[b], vns[b]
                vA8, vB8 = v8s[b]
                qt = qTs[b][:, ib * TB:(ib + 1) * TB]
                pv = ps_pv.tile([128, 2, TB], F32, tag="pv", name="pv")

                # off-diagonal kv tiles: fp8 probs, DoubleRow PV (2 kv tiles/mm)
                for pr in range(ib * 2):
                    jt0 = pr * 2
                    e2 = exp_pool.tile([128, 2, 2, TB], F8, tag="e2", name="e2")
                    for j in range(2):
                        jsl = slice((jt0 + j) * 128, (jt0 + j + 1) * 128)
                        s = ps_s.tile([128, 2, TB], F32, tag="s", name="s")
                        nc.tensor.matmul(
                            s[:, 0, :], lhsT=kT[0:64, jsl], rhs=qt[0:64, :],
                            start=True, stop=True, tile_position=(0, 0),
                        )
                        nc.tensor.matmul(
                            s[:, 1, :], lhsT=kT[64:128, jsl], rhs=qt[64:128, :],
                            start=True, stop=True, tile_position=(64, 0),
                        )
                        with nc.allow_low_precision(reason="off-diag probs in fp8"):
                            nc.scalar.activation(
                                e2[:, j, :, :], s[:],
                                mybir.ActivationFunctionType.Exp,
                                bias=ebias[:], scale=SCALE,
                            )
                    nc.tensor.matmul(
                        pv[0:128, 0, :], lhsT=vA8[:, pr, :, :],
                        rhs=e2[:, :, 0, :],
                        start=(pr == 0), stop=False,
                        perf_mode=mybir.MatmulPerfMode.DoubleRowSwInterleave,
                        skip_group_check=True,
                    )
                    nc.tensor.matmul(
                        pv[0:128, 1, :], lhsT=vB8[:, pr, :, :],
                        rhs=e2[:, :, 1, :],
                        start=(pr == 0), stop=False,
                        perf_mode=mybir.MatmulPerfMode.DoubleRowSwInterleave,
                        skip_group_check=True,
                    )

                # diagonal kv tiles: bf16 probs, causal column trim + triangle mask
                for jl in range(4):
                    jt = ib * 4 + jl
                    c0 = jl * 128
                    jsl = slice(jt * 128, (jt + 1) * 128)
                    s = ps_s.tile([128, 2, TB], F32, tag="s", name="sd")
                    nc.tensor.matmul(
                        s[:, 0, c0:TB], lhsT=kT[0:64, jsl], rhs=qt[0:64, c0:TB],
                        start=True, stop=True, tile_position=(0, 0),
                    )
                    nc.tensor.matmul(
                        s[:, 1, c0:TB], lhsT=kT[64:128, jsl], rhs=qt[64:128, c0:TB],
                        start=True, stop=True, tile_position=(64, 0),
                    )
                    e = exp_pool.tile([128, 2, TB], BF16, tag="e", name="e")
                    nc.scalar.activation(
                        e[:, :, c0:TB], s[:, :, c0:TB],
                        mybir.ActivationFunctionType.Exp,
                        bias=ebias[:], scale=SCALE,
                    )
                    with nc.allow_low_precision(reason="exact 0/1 mask on bf16 probs"):
                        nc.vector.tensor_mul(
                            e[:, 0, c0:c0 + 128], e[:, 0, c0:c0 + 128], tri_sb[:])
                        nc.vector.tensor_mul(
                            e[:, 1, c0:c0 + 128], e[:, 1, c0:c0 + 128], tri_sb[:])
                    first = (ib == 0 and jl == 0)
                    nc.tensor.matmul(
                        pv[0:65, 0, c0:TB], lhsT=v_nat[:, jt, 0:65],
                        rhs=e[:, 0, c0:TB],
                        start=first, stop=(jl == 3), skip_group_check=True,
                    )
                    nc.tensor.matmul(
                        pv[0:65, 1, c0:TB], lhsT=v_nat[:, jt, 65:130],
                        rhs=e[:, 1, c0:TB],
                        start=first, stop=(jl == 3), skip_group_check=True,
                    )

                # normalize both heads and ship to the AG input for this half
                blk = half * 8 + b * 2 + ib // 2
                srow = sr_pool.tile([1, 2, TB], F32, tag="sr", name="srow")
                nc.vector.tensor_copy(srow[:], pv[64:65, :, :])
                r32 = sr_pool.tile([1, 2, TB], F32, tag="sr", name="r32")
                nc.vector.reciprocal_approx_fast(out=r32[:], in_=srow[:])
                rb = evac_pool.tile([64, 2, TB], F32, tag="rb", name="rb")
                nc.gpsimd.partition_broadcast(rb[:], r32[:])
                outn = evac_pool.tile([64, 2, TB], BF16, tag="on", name="outn")
                with nc.allow_low_precision(reason="normalized attn out as bf16"):
                    nc.vector.tensor_mul(outn[:], pv[0:64, :, :], rb[:])
                chunk = b * 2 + ib // 2
                nc.sync.dma_start(out=a2i[half].ap()[chunk, 0:64, :], in_=outn[:, 0, :])
                nc.sync.dma_start(out=a2i[half].ap()[chunk, 64:128, :], in_=outn[:, 1, :])

            # ---- interleaved emission: qkv(b) | attnE(b) | attnO(b-1) ----
            for b in range(B):
                qkv_batch(b)
                if b == 0:
                    nc.sync.dma_start(
                        out=wproj_sb,
                        in_=wprojT.ap().rearrange("(kt p) m -> p kt m", p=128),
                    )
                for ib in (0, 2):
                    attn_block(b, ib, 0)
                if b >= 1:
                    for ib in (1, 3):
                        attn_block(b - 1, ib, 1)
            nc.gpsimd.collective_compute(
                "AllToAll", mybir.AluOpType.bypass,
                ins=[a2i[0].ap()], outs=[a2o[0].ap()],
                replica_groups=[list(range(N_CORES))],
            )
            for ib in (3, 1):
                attn_block(B - 1, ib, 1)

            # ---- output projection for my 1024-token slice ----
            for half in range(2):
                if half == 1:
                    nc.gpsimd.collective_compute(
                        "AllToAll", mybir.AluOpType.bypass,
                        ins=[a2i[1].ap()], outs=[a2o[1].ap()],
                        replica_groups=[list(range(N_CORES))],
                    )
                at = xt_pool.tile([128, NKT, TB], BF16, tag="at", name="at")
                for kt in range(NKT):
                    nc.sync.dma_start(
                        out=at[:, kt, :],
                        in_=a2o[half].ap()[kt, :, :],
                    )
                for dt in range(8):
                    ps = ps_g.tile([128, TB], F32, tag="g", name="psp")
                    for kt in range(NKT):
                        nc.tensor.matmul(
                            ps[:],
                            lhsT=wproj_sb[:, kt, dt * 128:(dt + 1) * 128],
                            rhs=at[:, kt, :],
                            start=(kt == 0),
                            stop=(kt == NKT - 1),
                        )
                    ot = po_pool.tile([128, TB], F32, tag="po", name="ot")
                    nc.scalar.activation(
                        ot[:], ps[:], mybir.ActivationFunctionType.Identity,
                        bias=bmat_sb[:, dt:dt + 1], scale=1.0,
                    )
                    nc.sync.dma_start(
                        out=outT.ap()[dt * 128:(dt + 1) * 128, half * TB:(half + 1) * TB],
                        in_=ot[:],
                    )

    nc.compile()
    return nc


_NC = None
_last_in_maps = None


def _get_nc():
    global _NC
    if _NC is None:
        _NC = build()
    return _NC


def kernel(x, w_qkv, w_proj, b_proj):
    nc = _get_nc()

    x = np.asarray(x, dtype=np.float32)
    w_qkv = np.asarray(w_qkv, dtype=np.float32)
    w_proj = np.asarray(w_proj, dtype=np.float32)
    b_proj = np.asarray(b_proj, dtype=np.float32)

    xT = np.ascontiguousarray(x.reshape(BT, C).T).astype(ml_dtypes.bfloat16)
    bmat = np.ascontiguousarray(b_proj.reshape(8, 128).T)
    p = np.arange(128)[:, None]
    f = np.arange(128)[None, :]
    tri = (p <= f).astype(ml_dtypes.bfloat16)

    # v features are computed reversed within each head so the fp8 DoubleRow
    # weight tiles (hw row = 127 - col) come out natural; the diag PV walks
    # v_nat with stride -1 to match. w_proj needs no permutation.
    wprojT = np.ascontiguousarray(w_proj.T).astype(ml_dtypes.bfloat16)

    in_maps = []
    for c in range(N_CORES):
        rows = slice(c * 128, (c + 1) * 128)
        w_v = w_qkv[2 * C:3 * C][rows]
        w_local = np.concatenate(
            [w_qkv[0:C][rows], w_qkv[C:2 * C][rows], w_v], axis=0
        )  # [384, C]
        in_maps.append({
            "xT": xT,
            "wqkvT": np.ascontiguousarray(w_local.T).astype(ml_dtypes.bfloat16),
            "wprojT": wprojT,
            "bmat": bmat,
            "tri": tri,
        })

    global _last_in_maps
    _last_in_maps = in_maps
    res = run_bass_kernel_spmd(nc, in_maps, core_ids=list(range(N_CORES)))
    outT_full = np.concatenate([res.results[c]["outT"] for c in range(N_CORES)], axis=1)
    return np.ascontiguousarray(outT_full.T).reshape(B, T, C)
